# revision 13
# baseline (speedup 1.0000x reference)
"""ChainKinematics Trainium2 kernel (8-core data-parallel), v2 fp16.

Math per batch element b:
  T_curr_i = offsets[i] @ Rz(theta[b, i])
  abs_i = abs_{i-1} @ T_curr_i           (abs_{-1} = I)
  rel_i = reset_i ? T_curr_i : rel_{i-1} @ T_curr_i

Device mapping (per core, 8192 batch elements):
  State S holds A (4x4 per batch elem) as S[k*32+g, r*256+bw] = A[g*256+bw, r, k]
  (column k on partition blocks of 32, row r in free dim), stored fp16.
  Step: U2 = [V0,V1,V1,V0] and U23 = [V2,V3] via block-diag fp16 lhsT on PE
  (V_j = A @ offsets[:, j]); DVE multiplies U2 by the trig tile (partition
  blocks [c, c, s, -s]) into fp16 pq; then either a fp16 DVE add folds
  pq blocks into new cols 0,1, or a PE block-sum matmul + Pool/ACT copy does
  (routed for engine balance); ScalarE copies U23 -> new cols 2,3.
  Trig: t = (theta + phi_p + 4pi) mod 2pi on DVE (one fused tensor_scalar),
  then Sin(scale_p * t + bias_p) on ACT keeps the LUT argument in [-pi, pi].
"""

import sys

sys.path.insert(0, "/opt/trn_rl_repo")

import numpy as np

N_BODIES = 32
BATCH = 65536
N_CORES = 8
BC = BATCH // N_CORES  # 8192 per core
G = 32  # batch groups (partition blocks)
BW = BC // G  # 256 batch per group
FH = 4 * BW  # 1024: free size of one chain-slot (r, bw)
TWO_PI = float(2 * np.pi)
PI = float(np.pi)

_cache = {}


def _build_program(resets):
    """Build the Bass program. resets: sorted tuple of rel-restart bodies (>0)."""
    from concourse import bass, mybir, tile, bacc

    f32 = mybir.dt.float32
    f16 = mybir.dt.float16

    split = resets[0] if resets else N_BODIES  # first dual body

    nc = bacc.Bacc(None, target_bir_lowering=False, debug=False)
    threp_d = nc.dram_tensor("threp", [128, BC], f32, kind="ExternalInput")
    wall_d = nc.dram_tensor("wall", [128, N_BODIES * 192], f16, kind="ExternalInput")
    wsum_d = nc.dram_tensor("wsum", [128, 64], f16, kind="ExternalInput")
    oabs_d = nc.dram_tensor("oabs", [N_BODIES, 128, FH], f16, kind="ExternalOutput")
    orel_d = nc.dram_tensor(
        "orel", [N_BODIES - split, 128, FH], f16, kind="ExternalOutput"
    )

    with tile.TileContext(nc) as tc:
        with (
            tc.tile_pool(name="wpool", bufs=1) as wpool,
            tc.tile_pool(name="trigpool", bufs=1) as trigpool,
            tc.tile_pool(name="cpool", bufs=1) as cpool,
        ):
            w_tile = wpool.tile([128, N_BODIES * 192], f16)
            nc.sync.dma_start(w_tile[:], wall_d[:])
            wsum = wpool.tile([128, 64], f16)
            nc.sync.dma_start(wsum[:], wsum_d[:])
            trig = trigpool.tile([128, BC], f32)

            # per-partition constants: m_b phase (cycles): c-blocks 0.25
            # (sin -> cos); Sin scale +-2pi maps cycle fraction t in
            # [-0.5, 0.5] to [-pi, pi] with the -s block sign folded in
            phi = cpool.tile([128, 1], f32)
            scl = cpool.tile([128, 1], f32)
            nc.vector.memset(phi[0:64, :], 0.25)
            nc.vector.memset(phi[64:128, :], 0.0)
            nc.vector.memset(scl[0:96, :], TWO_PI)
            nc.vector.memset(scl[96:128, :], -TWO_PI)

            # ---- trig phase (chunked so the chain can start early) ----
            # y1 = theta/2pi + m_b; y3 = round(y1) via the magic-number
            # round-trip; t = y1 - y3 in [-0.5, 0.5]; trig = Sin(+-2pi * t)
            with tc.tile_pool(name="scratch", bufs=1) as sp:
                threp = trigpool.tile([128, BC], f32, tag="threp")
                nc.sync.dma_start(threp[:], threp_d[:])
                MAGIC = float(1.5 * 2**23)
                bounds = [0, 2 * BW, 8 * BW, BC]
                for ci, (lo, hi) in enumerate(zip(bounds[:-1], bounds[1:])):
                    sl = slice(lo, hi)
                    n = hi - lo
                    y1 = sp.tile([128, n], f32, tag="y1")
                    nc.vector.tensor_scalar(
                        y1[:], threp[:, sl], float(1.0 / TWO_PI), phi[:, 0:1],
                        mybir.AluOpType.mult, mybir.AluOpType.add,
                    )
                    y2 = sp.tile([128, n], f32, tag="y2")
                    nc.vector.tensor_scalar(
                        y2[:], y1[:], MAGIC, None, mybir.AluOpType.add
                    )
                    y3 = sp.tile([128, n], f32, tag="y3")
                    eng_y3 = nc.vector if ci == 0 else nc.gpsimd
                    eng_y3.tensor_scalar(
                        y3[:], y2[:], MAGIC, None, mybir.AluOpType.subtract
                    )
                    y4 = sp.tile([128, n], f32, tag="y4")
                    eng_t = nc.vector if ci == 0 else nc.gpsimd
                    eng_t.tensor_tensor(
                        y4[:], y1[:], y3[:], mybir.AluOpType.subtract
                    )
                    nc.scalar.activation(
                        trig[:, sl], y4[:], mybir.ActivationFunctionType.Sin,
                        bias=0.0, scale=scl[:, 0:1],
                    )

            # ---- state phase ----
            with (
                tc.tile_pool(name="spool", bufs=6) as spool,
                tc.tile_pool(name="idpool", bufs=1) as idpool,
                tc.tile_pool(name="mixpool", bufs=10) as mixpool,
            ):
                sid_f = idpool.tile([128, FH], f32)
                nc.vector.memset(sid_f[:], 0.0)
                for k in range(4):
                    nc.vector.memset(
                        sid_f[k * 32 : (k + 1) * 32, k * BW : (k + 1) * BW], 1.0
                    )
                sid = idpool.tile([128, FH], f16)
                nc.vector.tensor_copy(sid[:], sid_f[:])

                state = {"prev": None, "half": 0}

                def do_chunk(i, rhs_t, ro, s_next, fo, wd, w2, sub, SUB,
                             u2pool, uspool):
                    """One free-chunk of a chain step.

                    u2 (dup blocks [V0,V1,V1,V0]) -> trig mul -> pq;
                    us holds [c01 | u23] in one PSUM tile: u23-mm writes
                    parts 64-127, wsum-mm writes parts 0-63, then a single
                    128-partition copy finalizes the new state chunk.
                    pool_route: col0/1 instead via fp16 SBUF add on GPSIMD.
                    """
                    nr = SUB // BW
                    u2 = u2pool.tile([128, SUB], f32, tag="u2")
                    us = uspool.tile([128, SUB], f32, tag="us")
                    # matmul outputs are limited to one PSUM bank (512 f32)
                    for ch in range(0, SUB, 512):
                        cs = slice(ch, ch + 512)
                        rs = rhs_t[:, ro + ch : ro + ch + 512]
                        nc.tensor.matmul(
                            u2[:, cs], wd, rs, start=True, stop=True
                        )
                        nc.tensor.matmul(
                            us[64:128, cs], w2, rs, start=True, stop=True
                        )
                    tsl = slice(i * BW, (i + 1) * BW)
                    tb = trig[:, tsl].unsqueeze(1).broadcast_to([128, nr, BW])
                    pq = mixpool.tile([128, SUB], f16, tag="pq")
                    nc.vector.tensor_mul(
                        pq[:].rearrange("p (r b) -> p r b", b=BW),
                        u2[:].rearrange("p (r b) -> p r b", b=BW),
                        tb,
                    )
                    h = state["half"]
                    state["half"] += 1
                    lo = fo + sub
                    for ch in range(0, SUB, 512):
                        cs = slice(ch, ch + 512)
                        nc.tensor.matmul(
                            us[0:64, cs], wsum[:], pq[:, cs],
                            start=True, stop=True,
                        )
                    if h % 2 == 0:
                        nc.scalar.copy(s_next[:, lo : lo + SUB], us[:])
                    else:
                        nc.vector.tensor_copy(s_next[:, lo : lo + SUB], us[:])

                # singles: bodies [0, split), pipeline depth from 2 sub-halves
                with (
                    tc.tile_pool(name="u2s", bufs=3, space=bass.MemorySpace.PSUM) as u2s,
                    tc.tile_pool(name="uss", bufs=3, space=bass.MemorySpace.PSUM) as uss,
                ):
                    for i in range(split):
                        s_next = spool.tile([128, 2 * FH], f16, tag="state")
                        rhs_t = sid if i == 0 else state["prev"]
                        wd = w_tile[:, i * 192 : i * 192 + 128]
                        w2 = w_tile[:, i * 192 + 128 : i * 192 + 192]
                        for sub in range(0, FH, 512):
                            do_chunk(
                                i, rhs_t, sub, s_next, 0,
                                wd, w2, sub, 512, u2s, uss,
                            )
                        nc.sync.dma_start(oabs_d[i, :, :], s_next[:, 0:FH])
                        state["prev"] = s_next

                # duals: bodies [split, N), full-slot chunks
                with (
                    tc.tile_pool(name="u2d", bufs=2, space=bass.MemorySpace.PSUM) as u2d,
                    tc.tile_pool(name="usd", bufs=2, space=bass.MemorySpace.PSUM) as usd,
                ):
                    for i in range(split, N_BODIES):
                        s_next = spool.tile([128, 2 * FH], f16, tag="state")
                        wd = w_tile[:, i * 192 : i * 192 + 128]
                        w2 = w_tile[:, i * 192 + 128 : i * 192 + 192]
                        for slot in (0, 1):
                            if slot == 1 and i in resets:
                                rhs_t, ro = sid, 0
                            else:
                                ro = FH if (slot == 1 and i > split) else 0
                                rhs_t = state["prev"]
                            fo = slot * FH
                            do_chunk(
                                i, rhs_t, ro, s_next, fo,
                                wd, w2, 0, FH, u2d, usd,
                            )
                            if slot == 0:
                                nc.sync.dma_start(
                                    oabs_d[i, :, :], s_next[:, 0:FH]
                                )
                            else:
                                nc.sync.dma_start(
                                    orel_d[i - split, :, :],
                                    s_next[:, FH : 2 * FH],
                                )
                        state["prev"] = s_next

    nc.compile()
    return nc, split


def kernel(theta, offsets, reset_mask):
    theta = np.asarray(theta, dtype=np.float32)
    offsets = np.asarray(offsets, dtype=np.float32)
    reset_mask = np.asarray(reset_mask)
    assert theta.shape == (BATCH, N_BODIES)
    assert bool(reset_mask[0]), "chain must reset at body 0"
    resets = tuple(int(i) for i in np.flatnonzero(reset_mask) if i > 0)

    from concourse.bass_utils import run_bass_kernel_spmd

    key = resets
    if key not in _cache:
        _cache[key] = _build_program(resets)
    nc, split = _cache[key]

    # block-sum lhsT: col0 = PQ0 + PQ2, col1 = PQ1 + PQ3
    W_sum = np.zeros((128, 64), np.float16)
    for q, j in [(0, 0), (2, 0), (1, 1), (3, 1)]:
        W_sum[q * G + np.arange(G), j * G + np.arange(G)] = 1.0
    # host-prepared weights: per body, lhsT blocks for [u0,u1,u1,u0] and [u2,u3]
    W_all = np.zeros((128, N_BODIES * 192), np.float16)
    gidx = np.arange(G)
    for i in range(N_BODIES):
        O = offsets[i]
        for k in range(4):
            for mb, j in enumerate([0, 1, 1, 0]):
                W_all[k * G + gidx, i * 192 + mb * G + gidx] = O[k, j]
            for mb, j in enumerate([2, 3]):
                W_all[k * G + gidx, i * 192 + 128 + mb * G + gidx] = O[k, j]

    # host-prepared theta: [128, BC] with partition blocks [c,c,s,-s] all equal
    # to theta in layout [g, (i, bw)]; value th[g*BW+bw, i] at (q*32+g, i*BW+bw)
    in_maps = []
    for c in range(N_CORES):
        thc = theta[c * BC : (c + 1) * BC]  # [8192, 32]
        th_g = np.ascontiguousarray(
            thc.reshape(G, BW, N_BODIES).transpose(0, 2, 1).reshape(G, BW * N_BODIES)
        )  # [32, 8192]
        threp = np.tile(th_g, (4, 1))  # [128, 8192]
        in_maps.append({"threp": threp, "wall": W_all, "wsum": W_sum})

    out = run_bass_kernel_spmd(nc, in_maps, core_ids=list(range(N_CORES)))
    kernel.last_exec_ns = out.exec_time_ns
    kernel.last_results = out

    def decode(arr):
        # [nb, 128, FH] -> [nb, BC, 4, 4]: p=(k,g), f=(r,bw)
        nb = arr.shape[0]
        a = np.asarray(arr, dtype=np.float32).reshape(nb, 4, G, 4, BW)  # i,k,g,r,bw
        return np.ascontiguousarray(
            a.transpose(0, 2, 4, 3, 1).reshape(nb, BC, 4, 4)
        )

    abs_full = np.empty((N_BODIES, BATCH, 4, 4), np.float32)
    rel_full = np.empty((N_BODIES, BATCH, 4, 4), np.float32)
    for c in range(N_CORES):
        res = out.results[c]
        bsl = slice(c * BC, (c + 1) * BC)
        abs_full[:, bsl] = decode(res["oabs"])
        rel_full[split:, bsl] = decode(res["orel"])
    rel_full[:split] = abs_full[:split]
    return abs_full, rel_full


kernel.last_exec_ns = None
kernel.last_results = None
